# revision 1
# baseline (speedup 1.0000x reference)
"""CenterLoss Trainium2 kernel.

reference semantics:
    feats  = features.reshape(4096, 96)
    label  = argmax(predicts.reshape(4096, 6625), axis=1)   # first occurrence
    d[n]   = ||feats[n] - centers[label[n]]||^2
    loss   = (sum_n clip(d[n], 1e-12, 1e12) + (4096*6625-4096)*1e-12) / 4096

Sharding: data-parallel over the flattened 4096-row batch axis, 512 rows per
core across 8 cores; centers replicated. Each core returns 128 per-partition
distance sums; the host does the final (tiny) reduction ("all-reduce").

Per-core pipeline (phases batched over the 4 row-tiles of 128 rows each so
engines never stall on each other's in-order queues):
  A. stream predicts tiles [128, 6625] HBM->SBUF in quarter-DMAs
     (the memory-bound part, ~13.6 MB/core) + one features DMA
  B. per-chunk max-reduce over [128, 25, 265] views -> cmax [128, 4*25];
     row max m4 [128, 4]; first-max chunk id via
     reduce_min(is_ge(cmax, m4) * (j - 25)) — batched over all 4 tiles
  C. indirect-DMA gather of each row's winning 265-wide chunk; position
     within chunk via the same is_ge/iota/reduce_min trick;
     label = 265*chunk + pos; indirect-DMA gather of centers[label]
  D. acc[p] = sum over tiles/dims of (f - c)^2 via one gpsimd subtract +
     one ACT Square with accumulate
All argmax stages use first-occurrence tie-breaking, matching jnp.argmax
bit-exactly.
"""

import numpy as np

NUM_CLASSES = 6625
FEAT_DIM = 96
N_ROWS = 4096           # B*T = 64*64
N_CORES = 8
ROWS_PER_CORE = N_ROWS // N_CORES   # 512
P = 128                 # partitions
N_TILES = ROWS_PER_CORE // P        # 4 row-tiles per core
CH = 265                # chunk size (6625 = 25 * 265)
NCHUNK = NUM_CLASSES // CH          # 25
OUT_COLS = 1            # per-partition distance sums

_CACHE = {}


def _build_nc(reps=1, ablate="full", nsplit=8):
    if ("nc", reps, ablate, nsplit) in _CACHE:
        return _CACHE[("nc", reps, ablate, nsplit)]

    from contextlib import ExitStack

    import concourse.bass as bass
    import concourse.tile as tile
    from concourse import bacc, mybir

    nc = bacc.Bacc(
        "TRN2",
        target_bir_lowering=False,
        debug=False,
        num_devices=N_CORES,
    )

    predicts = nc.dram_tensor(
        "predicts", [ROWS_PER_CORE, NUM_CLASSES], mybir.dt.float32,
        kind="ExternalInput",
    )
    features = nc.dram_tensor(
        "features", [ROWS_PER_CORE, FEAT_DIM], mybir.dt.float32,
        kind="ExternalInput",
    )
    centers = nc.dram_tensor(
        "centers", [NUM_CLASSES, FEAT_DIM], mybir.dt.float32,
        kind="ExternalInput",
    )
    out = nc.dram_tensor(
        "out", [P, OUT_COLS], mybir.dt.float32, kind="ExternalOutput",
    )

    fadd = mybir.AluOpType.add
    fmul = mybir.AluOpType.mult

    with tile.TileContext(nc) as tc:
        with ExitStack() as ctx:
            xpool = ctx.enter_context(tc.tile_pool(name="x", bufs=4))
            small = ctx.enter_context(tc.tile_pool(name="small", bufs=5))
            const = ctx.enter_context(tc.tile_pool(name="const", bufs=1))

            # negrevj25[p, t, j] = j - 25   (argmin picks first max pos)
            nrj25_i = const.tile([P, N_TILES * NCHUNK], mybir.dt.int32)
            nc.gpsimd.iota(
                nrj25_i[:], pattern=[[0, N_TILES], [1, NCHUNK]], base=-NCHUNK,
                channel_multiplier=0)
            nrj25 = const.tile([P, N_TILES * NCHUNK], mybir.dt.float32)
            nc.vector.tensor_copy(nrj25[:], nrj25_i[:])

            # negrevj265[p, t, j] = j - 265
            nrj265_i = const.tile([P, N_TILES * CH], mybir.dt.int32)
            nc.gpsimd.iota(
                nrj265_i[:], pattern=[[0, N_TILES], [1, CH]], base=-CH,
                channel_multiplier=0)
            nrj265 = const.tile([P, N_TILES * CH], mybir.dt.float32)
            nc.vector.tensor_copy(nrj265[:], nrj265_i[:])

            # prebase[p, t] = (t*128 + p)*25 + 25
            pb4_i = const.tile([P, N_TILES], mybir.dt.int32)
            nc.gpsimd.iota(
                pb4_i[:], pattern=[[P * NCHUNK, N_TILES]], base=NCHUNK,
                channel_multiplier=NCHUNK)
            pb4 = const.tile([P, N_TILES], mybir.dt.float32)
            nc.vector.tensor_copy(pb4[:], pb4_i[:])

            # per-partition distance accumulator (summed over row tiles)
            acc = const.tile([P, 1], mybir.dt.float32)

            # warm the ACT Square table set while DMAs stream
            actwarm = const.tile([P, 1], mybir.dt.float32)
            nc.scalar.activation(
                actwarm[:], pb4[:, 0:1],
                mybir.ActivationFunctionType.Square)

            # predicts viewed as rows of 265 elements: [512*25, 265]
            pred_chunks = predicts.ap().rearrange("r (a b) -> (r a) b", b=CH)

            # chunk-count split per partial DMA/reduce
            QSPLIT = {2: [13, 12], 4: [7, 6, 6, 6],
                      8: [4, 3, 3, 3, 3, 3, 3, 3]}[nsplit]
            QOFF = [0]
            for q in QSPLIT:
                QOFF.append(QOFF[-1] + q)

            for _ in range(reps):
                # ---- phase A: stream predicts + features ----
                xs = []
                for t in range(N_TILES):
                    x = xpool.tile([P, NUM_CLASSES], mybir.dt.float32, tag="x")
                    for q in range(nsplit):
                        c0, c1 = QOFF[q] * CH, QOFF[q + 1] * CH
                        nc.sync.dma_start(
                            x[:, c0:c1],
                            predicts.ap()[t * P:(t + 1) * P, c0:c1])
                    xs.append(x)
                ftile = small.tile(
                    [P, N_TILES * FEAT_DIM], mybir.dt.float32, tag="feat")
                nc.sync.dma_start(
                    ftile[:],
                    features.ap().rearrange("(t p) d -> p t d", p=P))

                if ablate == "dma":
                    for t in range(N_TILES):
                        xv = xs[t][:].rearrange("p (a b) -> p a b", b=CH)
                        nc.vector.tensor_reduce(
                            acc[:, 0:1], xv[:, 0:1, :],
                            axis=mybir.AxisListType.XY, op=mybir.AluOpType.max)
                        nc.vector.tensor_reduce(
                            acc[:, 0:1], xv[:, NCHUNK - 1:NCHUNK, :],
                            axis=mybir.AxisListType.XY, op=mybir.AluOpType.max)
                    continue

                # ---- phase B: chunk maxes + batched level-1 argmax ----
                cmax4 = small.tile(
                    [P, N_TILES * NCHUNK], mybir.dt.float32, tag="cmax4")
                for t in range(N_TILES):
                    xv = xs[t][:].rearrange("p (a b) -> p a b", b=CH)
                    for q in range(nsplit):
                        nc.vector.tensor_reduce(
                            cmax4[:, t * NCHUNK + QOFF[q]:
                                  t * NCHUNK + QOFF[q + 1]],
                            xv[:, QOFF[q]:QOFF[q + 1], :],
                            axis=mybir.AxisListType.X, op=mybir.AluOpType.max)

                cm4v = cmax4[:].rearrange("p (t j) -> p t j", j=NCHUNK)
                m4 = small.tile([P, N_TILES], mybir.dt.float32, tag="m4")
                nc.vector.tensor_reduce(
                    m4[:], cm4v, axis=mybir.AxisListType.X,
                    op=mybir.AluOpType.max)

                if ablate == "noidx":
                    nc.vector.tensor_copy(acc[:, 0:1], m4[:, 0:1])
                    continue

                eq1 = small.tile(
                    [P, N_TILES * NCHUNK], mybir.dt.float32, tag="eq1")
                nc.vector.tensor_tensor(
                    out=eq1[:].rearrange("p (t j) -> p t j", j=NCHUNK),
                    in0=cm4v,
                    in1=m4[:][:, :, None].to_broadcast(
                        [P, N_TILES, NCHUNK]),
                    op=mybir.AluOpType.is_ge)
                nc.vector.tensor_tensor(
                    out=eq1[:], in0=eq1[:], in1=nrj25[:],
                    op=mybir.AluOpType.mult)
                r1 = small.tile([P, N_TILES], mybir.dt.float32, tag="r1")
                nc.vector.tensor_reduce(
                    r1[:], eq1[:].rearrange("p (t j) -> p t j", j=NCHUNK),
                    axis=mybir.AxisListType.X, op=mybir.AluOpType.min)

                # chunk-row id = (t*128+p)*25 + 25 + r1
                rsi4 = small.tile([P, N_TILES], mybir.dt.int32, tag="rsi4")
                nc.vector.tensor_tensor(
                    out=rsi4[:], in0=r1[:], in1=pb4[:], op=mybir.AluOpType.add)

                chunkcat = small.tile(
                    [P, N_TILES * CH], mybir.dt.float32, tag="chunkcat")
                for t in range(N_TILES):
                    nc.gpsimd.indirect_dma_start(
                        out=chunkcat[:, t * CH:(t + 1) * CH],
                        out_offset=None,
                        in_=pred_chunks,
                        in_offset=bass.IndirectOffsetOnAxis(
                            ap=rsi4[:, t:t + 1], axis=0))

                # ---- phase C: batched level-2 argmax + centers gather ----
                eq2 = small.tile(
                    [P, N_TILES * CH], mybir.dt.float32, tag="eq2")
                nc.vector.tensor_tensor(
                    out=eq2[:].rearrange("p (t j) -> p t j", j=CH),
                    in0=chunkcat[:].rearrange("p (t j) -> p t j", j=CH),
                    in1=m4[:][:, :, None].to_broadcast(
                        [P, N_TILES, CH]),
                    op=mybir.AluOpType.is_ge)
                nc.vector.tensor_tensor(
                    out=eq2[:], in0=eq2[:], in1=nrj265[:],
                    op=mybir.AluOpType.mult)
                r2 = small.tile([P, N_TILES], mybir.dt.float32, tag="r2")
                nc.vector.tensor_reduce(
                    r2[:], eq2[:].rearrange("p (t j) -> p t j", j=CH),
                    axis=mybir.AxisListType.X, op=mybir.AluOpType.min)

                # label = chunk*265 + pos = 265*r1 + r2 + 6890
                labt = small.tile([P, N_TILES], mybir.dt.float32, tag="labt")
                nc.vector.tensor_scalar(
                    labt[:], r1[:], float(CH), float(CH * NCHUNK + CH),
                    op0=fmul, op1=fadd)
                labi4 = small.tile([P, N_TILES], mybir.dt.int32, tag="labi4")
                nc.vector.tensor_tensor(
                    out=labi4[:], in0=labt[:], in1=r2[:],
                    op=mybir.AluOpType.add)

                cselcat = small.tile(
                    [P, N_TILES * FEAT_DIM], mybir.dt.float32, tag="cselcat")
                for t in range(N_TILES):
                    nc.gpsimd.indirect_dma_start(
                        out=cselcat[:, t * FEAT_DIM:(t + 1) * FEAT_DIM],
                        out_offset=None,
                        in_=centers.ap(),
                        in_offset=bass.IndirectOffsetOnAxis(
                            ap=labi4[:, t:t + 1], axis=0))

                # ---- phase D: acc[p] = sum_t sum_d (f - c)^2 ----
                diff = small.tile(
                    [P, N_TILES * FEAT_DIM], mybir.dt.float32, tag="diff")
                nc.gpsimd.tensor_sub(diff[:], ftile[:], cselcat[:])
                sq = small.tile(
                    [P, N_TILES * FEAT_DIM], mybir.dt.float32, tag="sq")
                nc.scalar.activation(
                    sq[:], diff[:], mybir.ActivationFunctionType.Square,
                    accum_out=acc[:, 0:1])

            nc.sync.dma_start(out.ap()[:, :], acc[:])

    nc.compile()
    _CACHE[("nc", reps, ablate, nsplit)] = nc
    return nc


def _build_null_nc():
    """Trivial NEFF (memset + tiny DMA out) to estimate launch overhead."""
    if "null" in _CACHE:
        return _CACHE["null"]

    from contextlib import ExitStack

    import concourse.tile as tile
    from concourse import bacc, mybir

    nc = bacc.Bacc(
        "TRN2", target_bir_lowering=False, debug=False, num_devices=N_CORES)
    predicts = nc.dram_tensor(
        "predicts", [ROWS_PER_CORE, NUM_CLASSES], mybir.dt.float32,
        kind="ExternalInput")
    features = nc.dram_tensor(
        "features", [ROWS_PER_CORE, FEAT_DIM], mybir.dt.float32,
        kind="ExternalInput")
    centers = nc.dram_tensor(
        "centers", [NUM_CLASSES, FEAT_DIM], mybir.dt.float32,
        kind="ExternalInput")
    out = nc.dram_tensor(
        "out", [P, OUT_COLS], mybir.dt.float32, kind="ExternalOutput")
    with tile.TileContext(nc) as tc:
        with ExitStack() as ctx:
            pool = ctx.enter_context(tc.tile_pool(name="p", bufs=1))
            acc = pool.tile([P, OUT_COLS], mybir.dt.float32)
            nc.vector.memset(acc[:], 0.0)
            nc.sync.dma_start(out.ap()[:, :], acc[:])
    nc.compile()
    _CACHE["null"] = nc
    return nc


def kernel(features, predicts, centers):
    from concourse.bass_utils import run_bass_kernel_spmd

    nc = _build_nc()

    feats = np.ascontiguousarray(
        np.asarray(features, dtype=np.float32).reshape(N_ROWS, FEAT_DIM))
    preds = np.ascontiguousarray(
        np.asarray(predicts, dtype=np.float32).reshape(N_ROWS, NUM_CLASSES))
    cents = np.ascontiguousarray(np.asarray(centers, dtype=np.float32))

    in_maps = []
    for m in range(N_CORES):
        s = slice(m * ROWS_PER_CORE, (m + 1) * ROWS_PER_CORE)
        in_maps.append({
            "predicts": np.ascontiguousarray(preds[s]),
            "features": np.ascontiguousarray(feats[s]),
            "centers": cents,
        })

    res = run_bass_kernel_spmd(nc, in_maps, core_ids=list(range(N_CORES)))

    d = np.concatenate([r["out"].reshape(-1) for r in res.results])
    d = np.clip(d.astype(np.float64), 1e-12, 1e12)
    total = d.sum() + (N_ROWS * NUM_CLASSES - N_ROWS) * 1e-12
    return np.asarray(total / N_ROWS, dtype=np.float32)

